# revision 6
# baseline (speedup 1.0000x reference)
"""Trainium2 Bass kernel for conv->conv->self-attention->pool->fc classifier.

Shards batch 256 across 8 NeuronCores (32 samples each), weights replicated.
Heavy algebraic folding is done host-side (see _prep_consts); the device code
per sample is: 2 matmul convs + relu, one 65x65 "score operator" matmul,
4 QK^T score matmuls, exp with fused row-sum accumulation, reciprocal, a
recip-weighted column-sum matvec (w = E^T r), and a 10-wide output matvec.
The attn@V matmul is eliminated entirely: mean-pooling commutes with
attention, so only attention column sums (w) are needed.
"""
import sys

sys.path.insert(0, "/opt/trn_rl_repo")

import numpy as np

import concourse.bass as bass
import concourse.tile as tile
from concourse import bacc, mybir
from concourse.bass_utils import run_bass_kernel_spmd

# Problem constants (hardcoded per harness contract)
B, C_IN, L, NCLASS = 256, 6, 512, 10
NCORES = 8
BS = B // NCORES          # samples per core
C1, C2 = 32, 64           # conv output channels
K1 = 3 * C_IN + 1         # 19: im2col rows + ones row
DA = C2 + 1               # 65: augmented feature dim
DT = mybir.dt.float32
EPS = 1e-5


def _prep_consts(p):
    """Fold all weights/biases/BN into the minimal set of device tensors."""
    inv1 = p["bn1_g"] / np.sqrt(p["bn1_v"] + EPS)            # [32]
    b1p = p["conv1_b"] * inv1 + p["bn1_b"] - p["bn1_m"] * inv1
    # W1p [19, 32]: rows t*6+c hold conv1_w[o,c,t]*inv1[o]; row 18 = fused bias
    w1p = np.zeros((K1, C1), np.float32)
    for t in range(3):
        w1p[t * C_IN:(t + 1) * C_IN, :] = (
            p["conv1_w"][:, :, t] * inv1[:, None]).T
    w1p[K1 - 1, :] = b1p

    inv2 = p["bn2_g"] / np.sqrt(p["bn2_v"] + EPS)            # [64]
    b2p = (p["conv2_b"] * inv2 + p["bn2_b"] - p["bn2_m"] * inv2).astype(
        np.float32).reshape(C2, 1)
    # W2 [32, 3*64]: column block t holds conv2 tap t (BN2-scaled), lhsT layout
    w2 = np.concatenate([(p["conv2_w"][:, :, t] * inv2[:, None]).T
                         for t in range(3)], axis=1).astype(np.float32)

    wq, bq, wk, bk = p["wq"], p["bq"], p["wk"], p["bk"]
    m_blk = wq.T @ wk                                        # [64, 64]
    a_vec = wq.T @ bk                                        # [64]
    b_vec = wk.T @ bq                                        # [64]
    c_sc = float(bq @ bk)
    maug = np.zeros((DA, DA), np.float32)
    maug[:C2, :C2] = m_blk
    maug[:C2, C2] = a_vec
    maug[C2, :C2] = b_vec
    maug[C2, C2] = c_sc
    maug /= np.sqrt(64.0)
    maug_t = np.ascontiguousarray(maug.T)                    # lhsT for t-matmul

    # FaugT [65, 10]: G_t[k,c] = h_aug(k) . FaugT[:,c]
    #   rows 0-63 = (fc_w @ wv / 512).T ; row 64 = (fc_w @ bv + fc_b)/512
    # (row 64 exploits sum_k w[k] == 512 up to fp eps)
    faug_t = np.zeros((DA, NCLASS), np.float32)
    faug_t[:C2, :] = (p["fc_w"] @ p["wv"] / L).T
    faug_t[C2, :] = (p["fc_w"] @ p["bv"] + p["fc_b"]) / L
    return {
        "w1p": w1p.astype(np.float32),
        "w2": w2,
        "b2p": b2p,
        "maug_t": maug_t,
        "faug_t": faug_t,
    }


def _prep_x3(x_shard):
    """im2col with ones row: [BS,6,512] -> [19, BS*512] (fp32)."""
    bs = x_shard.shape[0]
    x3 = np.zeros((K1, bs, L), np.float32)
    x3[0:C_IN, :, 1:] = np.transpose(x_shard, (1, 0, 2))[:, :, :-1]
    x3[C_IN:2 * C_IN, :, :] = np.transpose(x_shard, (1, 0, 2))
    x3[2 * C_IN:3 * C_IN, :, :511] = np.transpose(x_shard, (1, 0, 2))[:, :, 1:]
    x3[K1 - 1, :, :] = 1.0
    return np.ascontiguousarray(x3.reshape(K1, bs * L))


def _build_program():
    nc = bacc.Bacc("TRN2", target_bir_lowering=False, debug=False,
                   enable_asserts=True)
    x3_d = nc.dram_tensor("x3", [K1, BS * L], DT, kind="ExternalInput")
    w1p_d = nc.dram_tensor("w1p", [K1, C1], DT, kind="ExternalInput")
    w2_d = nc.dram_tensor("w2", [C1, 3 * C2], DT, kind="ExternalInput")
    b2p_d = nc.dram_tensor("b2p", [C2, 1], DT, kind="ExternalInput")
    maug_d = nc.dram_tensor("maug_t", [DA, DA], DT, kind="ExternalInput")
    faug_d = nc.dram_tensor("faug_t", [DA, NCLASS], DT, kind="ExternalInput")
    out_d = nc.dram_tensor("out", [1, BS * NCLASS], DT, kind="ExternalOutput")

    with tile.TileContext(nc) as tc:
        with (
            tc.tile_pool(name="consts", bufs=1) as consts,
            tc.tile_pool(name="sb", bufs=2) as sb,
            tc.tile_pool(name="epool", bufs=8) as epool,
            tc.tile_pool(name="small", bufs=4) as small,
            tc.tile_pool(name="ps", bufs=1, space="PSUM") as ps,
            tc.tile_pool(name="ps_s", bufs=2, space="PSUM") as ps_s,
        ):
            x3_t = consts.tile([K1, BS * L], DT)
            w1p_t = consts.tile([K1, C1], DT)
            w2_t = consts.tile([C1, 3 * C2], DT)
            b2p_t = consts.tile([C2, 1], DT)
            maug_t = consts.tile([DA, DA], DT)
            faug_t = consts.tile([DA, NCLASS], DT)
            out_row = consts.tile([1, BS * NCLASS], DT)
            nc.sync.dma_start(x3_t[:], x3_d.ap())
            nc.sync.dma_start(w1p_t[:], w1p_d.ap())
            nc.sync.dma_start(w2_t[:], w2_d.ap())
            nc.sync.dma_start(b2p_t[:], b2p_d.ap())
            nc.sync.dma_start(maug_t[:], maug_d.ap())
            nc.sync.dma_start(faug_t[:], faug_d.ap())

            for s in range(BS):
                # conv1 (+BN1 fused, bias via ones row) -> relu
                c1_p = ps.tile([C1, L], DT, tag="c1")
                nc.tensor.matmul(c1_p[:], w1p_t[:],
                                 x3_t[:, s * L:(s + 1) * L],
                                 start=True, stop=True)
                h1pad = sb.tile([C1, L + 2], DT, tag="h1")
                nc.gpsimd.memset(h1pad[:, 0:1], 0.0)
                nc.gpsimd.memset(h1pad[:, L + 1:L + 2], 0.0)
                nc.vector.tensor_scalar_max(h1pad[:, 1:L + 1], c1_p[:], 0.0)

                # conv2 (+BN2 fused) as 3 shifted accumulating matmuls
                c2_p = ps.tile([C2, L], DT, tag="c2")
                for t in range(3):
                    nc.tensor.matmul(c2_p[:],
                                     w2_t[:, t * C2:(t + 1) * C2],
                                     h1pad[:, t:t + L],
                                     start=(t == 0), stop=(t == 2))
                h2aug = sb.tile([DA, L], DT, tag="h2")
                nc.vector.tensor_scalar(
                    out=h2aug[0:C2, :], in0=c2_p[:], scalar1=b2p_t[:],
                    scalar2=0.0, op0=mybir.AluOpType.add,
                    op1=mybir.AluOpType.max)
                nc.gpsimd.memset(h2aug[C2:DA, :], 1.0)

                # t = Maug @ h2aug  (scores operator, all QK biases folded)
                t_p = ps.tile([DA, L], DT, tag="tp")
                nc.tensor.matmul(t_p[:], maug_t[:], h2aug[:],
                                 start=True, stop=True)
                t_s = sb.tile([DA, L], DT, tag="ts")
                nc.vector.tensor_copy(t_s[:], t_p[:])

                # S chunks + G_t (shares lhsT with S), exp with fused row-sums
                zcol = small.tile([128, 4], DT, tag="z")
                g_p = ps.tile([128, 4 * NCLASS], DT, tag="gp")
                e_ts = []
                for m in range(4):
                    s_p = ps_s.tile([128, L], DT, tag="sp")
                    lhs = h2aug[:, m * 128:(m + 1) * 128]
                    nc.tensor.matmul(s_p[:], lhs, t_s[:],
                                     start=True, stop=True)
                    nc.tensor.matmul(g_p[:, m * NCLASS:(m + 1) * NCLASS],
                                     lhs, faug_t[:], start=True, stop=True)
                    e_t = epool.tile([128, L], DT, tag="e")
                    nc.scalar.activation(
                        e_t[:], s_p[:], mybir.ActivationFunctionType.Exp,
                        accum_out=zcol[:, m:m + 1])
                    e_ts.append(e_t)

                rcol = small.tile([128, 4], DT, tag="r")
                nc.vector.reciprocal(rcol[:], zcol[:])

                # w = E^T r  (attention column sums, normalized)
                w_p = ps.tile([1, L], DT, tag="wp")
                for m in range(4):
                    nc.tensor.matmul(w_p[:], rcol[:, m:m + 1], e_ts[m][:],
                                     start=(m == 0), stop=(m == 3))
                w_s = small.tile([1, L], DT, tag="ws")
                nc.vector.tensor_copy(w_s[:], w_p[:])
                g_s = small.tile([128, 4 * NCLASS], DT, tag="gs")
                nc.vector.tensor_copy(g_s[:], g_p[:])

                # transpose w to partitions, then logits = w^T @ G_t
                w_t = small.tile([128, 4], DT, tag="wt")
                for m in range(4):
                    nc.sync.dma_start(w_t[:, m:m + 1],
                                      w_s[0:1, m * 128:(m + 1) * 128])
                lg_p = ps.tile([1, NCLASS], DT, tag="lp")
                for m in range(4):
                    nc.tensor.matmul(
                        lg_p[:], w_t[:, m:m + 1],
                        g_s[:, m * NCLASS:(m + 1) * NCLASS],
                        start=(m == 0), stop=(m == 3))
                nc.vector.tensor_copy(
                    out_row[0:1, s * NCLASS:(s + 1) * NCLASS], lg_p[:])

            nc.sync.dma_start(out_d.ap(), out_row[:])

    nc.compile()
    return nc


_NC_CACHE = {}


def _get_program():
    if "nc" not in _NC_CACHE:
        _NC_CACHE["nc"] = _build_program()
    return _NC_CACHE["nc"]


def kernel(**inputs):
    inputs = {k: np.asarray(v) for k, v in inputs.items()}
    consts = _prep_consts(inputs)
    x = inputs["x"].astype(np.float32)

    nc = _get_program()
    in_maps = []
    for i in range(NCORES):
        m = {"x3": _prep_x3(x[i * BS:(i + 1) * BS])}
        m.update({
            "w1p": consts["w1p"],
            "w2": consts["w2"],
            "b2p": consts["b2p"],
            "maug_t": consts["maug_t"],
            "faug_t": consts["faug_t"],
        })
        in_maps.append(m)
    res = run_bass_kernel_spmd(nc, in_maps, list(range(NCORES)))
    outs = [res.results[i]["out"].reshape(BS, NCLASS) for i in range(NCORES)]
    return np.concatenate(outs, axis=0)


# revision 12
# speedup vs baseline: 13.6505x; 13.6505x over previous
"""Trainium2 Bass kernel for conv->conv->self-attention->pool->fc classifier.

Shards batch 256 across 8 NeuronCores (32 samples each), weights replicated.
Heavy algebraic folding is done host-side (see _prep_consts); the device code
per sample is: 2 matmul convs + relu, one 65x65 "score operator" matmul,
4 QK^T score matmuls, exp with fused row-sum accumulation, reciprocal, a
recip-weighted column-sum matvec (w = E^T r), and a 10-wide output matvec.
The attn@V matmul is eliminated entirely: mean-pooling commutes with
attention, so only attention column sums (w) are needed.
"""
import sys

sys.path.insert(0, "/opt/trn_rl_repo")

import numpy as np

import concourse.bass as bass
import concourse.tile as tile
from concourse import bacc, mybir
from concourse.bass_utils import run_bass_kernel_spmd

# Problem constants (hardcoded per harness contract)
B, C_IN, L, NCLASS = 256, 6, 512, 10
NCORES = 8
BS = B // NCORES          # samples per core
C1, C2 = 32, 64           # conv output channels
K1 = 3 * C_IN + 1         # 19: im2col rows + ones row
DA = C2 + 1               # 65: augmented feature dim
DT = mybir.dt.float32
EPS = 1e-5


def _prep_consts(p):
    """Fold all weights/biases/BN into the minimal set of device tensors."""
    inv1 = p["bn1_g"] / np.sqrt(p["bn1_v"] + EPS)            # [32]
    b1p = p["conv1_b"] * inv1 + p["bn1_b"] - p["bn1_m"] * inv1
    # W1p [19, 32]: rows t*6+c hold conv1_w[o,c,t]*inv1[o]; row 18 = fused bias
    w1p = np.zeros((K1, C1), np.float32)
    for t in range(3):
        w1p[t * C_IN:(t + 1) * C_IN, :] = (
            p["conv1_w"][:, :, t] * inv1[:, None]).T
    w1p[K1 - 1, :] = b1p

    inv2 = p["bn2_g"] / np.sqrt(p["bn2_v"] + EPS)            # [64]
    b2p = (p["conv2_b"] * inv2 + p["bn2_b"] - p["bn2_m"] * inv2).astype(
        np.float32).reshape(C2, 1)
    # W2 [32, 3*64]: column block t holds conv2 tap t (BN2-scaled), lhsT layout
    w2 = np.concatenate([(p["conv2_w"][:, :, t] * inv2[:, None]).T
                         for t in range(3)], axis=1).astype(np.float32)

    wq, bq, wk, bk = p["wq"], p["bq"], p["wk"], p["bk"]
    m_blk = wq.T @ wk                                        # [64, 64]
    a_vec = wq.T @ bk                                        # [64]
    b_vec = wk.T @ bq                                        # [64]
    c_sc = float(bq @ bk)
    maug = np.zeros((DA, DA), np.float32)
    maug[:C2, :C2] = m_blk
    maug[:C2, C2] = a_vec
    maug[C2, :C2] = b_vec
    maug[C2, C2] = c_sc
    maug /= np.sqrt(64.0)
    maug_t = np.ascontiguousarray(maug.T)                    # lhsT for t-matmul

    # FaugT [65, 10]: G_t[k,c] = h_aug(k) . FaugT[:,c]
    #   rows 0-63 = (fc_w @ wv / 512).T ; row 64 = (fc_w @ bv + fc_b)/512
    # (row 64 exploits sum_k w[k] == 512 up to fp eps)
    faug_t = np.zeros((DA, NCLASS), np.float32)
    faug_t[:C2, :] = (p["fc_w"] @ p["wv"] / L).T
    faug_t[C2, :] = (p["fc_w"] @ p["bv"] + p["fc_b"]) / L
    return {
        "w1p": w1p.astype(np.float32),
        "w2": w2,
        "b2p": b2p,
        "maug_t": maug_t,
        "faug_t": faug_t,
    }


def _prep_x3(x_shard):
    """im2col with ones row: [BS,6,512] -> [19, BS*512] (fp32)."""
    bs = x_shard.shape[0]
    x3 = np.zeros((K1, bs, L), np.float32)
    x3[0:C_IN, :, 1:] = np.transpose(x_shard, (1, 0, 2))[:, :, :-1]
    x3[C_IN:2 * C_IN, :, :] = np.transpose(x_shard, (1, 0, 2))
    x3[2 * C_IN:3 * C_IN, :, :511] = np.transpose(x_shard, (1, 0, 2))[:, :, 1:]
    x3[K1 - 1, :, :] = 1.0
    return np.ascontiguousarray(x3.reshape(K1, bs * L))


def _build_program(repeat=1, dyn_loop=0):
    nc = bacc.Bacc("TRN2", target_bir_lowering=False, debug=False,
                   enable_asserts=True)
    x3_d = nc.dram_tensor("x3", [K1, BS * L], DT, kind="ExternalInput")
    w1p_d = nc.dram_tensor("w1p", [K1, C1], DT, kind="ExternalInput")
    w2_d = nc.dram_tensor("w2", [C1, 3 * C2], DT, kind="ExternalInput")
    b2p_d = nc.dram_tensor("b2p", [C2, 1], DT, kind="ExternalInput")
    maug_d = nc.dram_tensor("maug_t", [DA, DA], DT, kind="ExternalInput")
    faug_d = nc.dram_tensor("faug_t", [DA, NCLASS], DT, kind="ExternalInput")
    out_d = nc.dram_tensor("out", [1, BS * NCLASS], DT, kind="ExternalOutput")

    with tile.TileContext(nc) as tc:
        with (
            tc.tile_pool(name="consts", bufs=1) as consts,
            tc.tile_pool(name="sb", bufs=2) as sb,
            tc.tile_pool(name="epool", bufs=8) as epool,
            tc.tile_pool(name="small", bufs=4) as small,
            tc.tile_pool(name="ps", bufs=1, space="PSUM") as ps,
            tc.tile_pool(name="ps_s", bufs=2, space="PSUM") as ps_s,
        ):
            x3_t = consts.tile([K1, BS * L], DT)
            w1p_t = consts.tile([K1, C1], DT)
            w2_t = consts.tile([C1, 3 * C2], DT)
            b2p_t = consts.tile([C2, 1], DT)
            maug_t = consts.tile([DA, DA], DT)
            faug_t = consts.tile([DA, NCLASS], DT)
            out_row = consts.tile([1, BS * NCLASS], DT)
            nc.sync.dma_start(x3_t[:], x3_d.ap())
            nc.sync.dma_start(w1p_t[:], w1p_d.ap())
            nc.sync.dma_start(w2_t[:], w2_d.ap())
            nc.sync.dma_start(b2p_t[:], b2p_d.ap())
            nc.sync.dma_start(maug_t[:], maug_d.ap())
            nc.sync.dma_start(faug_t[:], faug_d.ap())

            import contextlib
            loop_cm = (tc.For_i(0, dyn_loop, 1) if dyn_loop
                       else contextlib.nullcontext())
            with loop_cm:
              for s in [s for _ in range(repeat) for s in range(BS)]:
                # conv1 (+BN1 fused, bias via ones row) -> relu
                c1_p = ps.tile([C1, L], DT, tag="c1")
                nc.tensor.matmul(c1_p[:], w1p_t[:],
                                 x3_t[:, s * L:(s + 1) * L],
                                 start=True, stop=True)
                h1pad = sb.tile([C1, L + 2], DT, tag="h1")
                nc.gpsimd.memset(h1pad[:, 0:1], 0.0)
                nc.gpsimd.memset(h1pad[:, L + 1:L + 2], 0.0)
                nc.vector.tensor_scalar_max(h1pad[:, 1:L + 1], c1_p[:], 0.0)

                # conv2 (+BN2 fused) as 3 shifted accumulating matmuls
                c2_p = ps.tile([C2, L], DT, tag="c2")
                for t in range(3):
                    nc.tensor.matmul(c2_p[:],
                                     w2_t[:, t * C2:(t + 1) * C2],
                                     h1pad[:, t:t + L],
                                     start=(t == 0), stop=(t == 2))
                h2aug = sb.tile([DA, L], DT, tag="h2")
                nc.vector.tensor_scalar(
                    out=h2aug[0:C2, :], in0=c2_p[:], scalar1=b2p_t[:],
                    scalar2=0.0, op0=mybir.AluOpType.add,
                    op1=mybir.AluOpType.max)
                nc.gpsimd.memset(h2aug[C2:DA, :], 1.0)

                # t = Maug @ h2aug  (scores operator, all QK biases folded)
                t_p = ps.tile([DA, L], DT, tag="tp")
                nc.tensor.matmul(t_p[:], maug_t[:], h2aug[:],
                                 start=True, stop=True)
                t_s = sb.tile([DA, L], DT, tag="ts")
                nc.vector.tensor_copy(t_s[:], t_p[:])

                # S chunks + G_t (shares lhsT with S), exp with fused row-sums
                zcol = small.tile([128, 4], DT, tag="z")
                g_p = ps.tile([128, 4 * NCLASS], DT, tag="gp")
                e_ts = []
                for m in range(4):
                    s_p = ps_s.tile([128, L], DT, tag="sp")
                    lhs = h2aug[:, m * 128:(m + 1) * 128]
                    nc.tensor.matmul(s_p[:], lhs, t_s[:],
                                     start=True, stop=True)
                    nc.tensor.matmul(g_p[:, m * NCLASS:(m + 1) * NCLASS],
                                     lhs, faug_t[:], start=True, stop=True)
                    e_t = epool.tile([128, L], DT, tag="e")
                    nc.scalar.activation(
                        e_t[:], s_p[:], mybir.ActivationFunctionType.Exp,
                        accum_out=zcol[:, m:m + 1])
                    e_ts.append(e_t)

                rcol = small.tile([128, 4], DT, tag="r")
                nc.vector.reciprocal(rcol[:], zcol[:])

                # w = E^T r  (attention column sums, normalized)
                w_p = ps.tile([1, L], DT, tag="wp")
                for m in range(4):
                    nc.tensor.matmul(w_p[:], rcol[:, m:m + 1], e_ts[m][:],
                                     start=(m == 0), stop=(m == 3))
                w_s = small.tile([1, L], DT, tag="ws")
                nc.vector.tensor_copy(w_s[:], w_p[:])
                g_s = small.tile([128, 4 * NCLASS], DT, tag="gs")
                nc.vector.tensor_copy(g_s[:], g_p[:])

                # transpose w to partitions, then logits = w^T @ G_t
                w_t = small.tile([128, 4], DT, tag="wt")
                for m in range(4):
                    nc.sync.dma_start(w_t[:, m:m + 1],
                                      w_s[0:1, m * 128:(m + 1) * 128])
                lg_p = ps.tile([1, NCLASS], DT, tag="lp")
                for m in range(4):
                    nc.tensor.matmul(
                        lg_p[:], w_t[:, m:m + 1],
                        g_s[:, m * NCLASS:(m + 1) * NCLASS],
                        start=(m == 0), stop=(m == 3))
                nc.vector.tensor_copy(
                    out_row[0:1, s * NCLASS:(s + 1) * NCLASS], lg_p[:])

            nc.sync.dma_start(out_d.ap(), out_row[:])

    nc.compile()
    return nc


_NC_CACHE = {}


def _get_program(repeat=1, dyn_loop=0):
    key = (repeat, dyn_loop)
    if key not in _NC_CACHE:
        _NC_CACHE[key] = _build_program(repeat, dyn_loop)
    return _NC_CACHE[key]


def kernel(**inputs):
    inputs = {k: np.asarray(v) for k, v in inputs.items()}
    consts = _prep_consts(inputs)
    x = inputs["x"].astype(np.float32)

    nc = _get_program()
    in_maps = []
    for i in range(NCORES):
        m = {"x3": _prep_x3(x[i * BS:(i + 1) * BS])}
        m.update({
            "w1p": consts["w1p"],
            "w2": consts["w2"],
            "b2p": consts["b2p"],
            "maug_t": consts["maug_t"],
            "faug_t": consts["faug_t"],
        })
        in_maps.append(m)
    res = run_bass_kernel_spmd(nc, in_maps, list(range(NCORES)))
    outs = [res.results[i]["out"].reshape(BS, NCLASS) for i in range(NCORES)]
    return np.concatenate(outs, axis=0)
